# revision 6
# baseline (speedup 1.0000x reference)
"""Multi-head attention Trainium2 Bass kernel (8 NeuronCores, SPMD).

Problem: B=4, S=2048, D=512, H=8 heads of DH=64.
  q = Q @ Wq[h].T ; k = K @ Wk[h].T ; v = V @ Wv[h].T     (per head)
  scores = q @ k.T / sqrt(DH)   (+ mask term: a per-query constant,
           which softmax is invariant to -> ignored)
  attn = softmax(scores, axis=keys)
  out  = concat_h(attn @ v) @ Wout.T

Sharding: core c handles batch b=c//2, query half qh=c%2 -> each core
computes a [1024, 512] slice of the output independently (no
collectives).

v2 design (baseline measured 188us; PE-busy model ~125us):
  - All matmul operands bf16; PSUM accumulation f32.
  - Scores matmuls (K=DH=64, 50% PE util) run as ROW-TILED HEAD PAIRS:
    head a of pair pr lives on partitions 0-63 of qT/kT, head b on
    64-127; consecutive matmuls at tile_position (0,0)/(64,0) execute
    CONCURRENTLY on disjoint row-groups of the PE array (HW-measured
    2.04x vs sequential) -> scores PE time halves.
  - Softmax exp split across engines: ScalarE computes exact exp for
    3/4 of (h,t) score tiles; the otherwise-idle DVE computes the
    remaining 1/4 via a Schraudolph bit-trick (one tensor_scalar:
    u16 = floor(scores*(128*log2e/8) + (127*128 - 6.9875)), bitcast
    bf16 ~= exp(scores/8), rel err ~1.8% RMS on those tiles; end-to-end
    rel err measured 1.2e-2 in numpy pipeline sim, threshold 2e-2).
  - ctx (attn @ v, M=65 with appended ones column accumulating softmax
    denominators) is split by query half: half0 [65,512] accumulates
    during the pair's slots 8-15 (4/slot), half1 re-reads the retained
    et tiles during the NEXT pair's slots 0-7.  This lets sc psum
    (2x[128,1024]) + ctx psum (2x[65,512]) + chunk psum (2x[128,512])
    fit exactly in the 8 PSUM banks with no double-buffer stalls.
  - Deferred chunk queue (q/k/v projections, output-projection
    partials) interleaves into the per-slot stream as PE filler, as in
    the baseline.
  - Output projection: partials of pairs 0+1 fold in one psum
    accumulation group (single DVE copy), pair 2 adds once, pair 3 +
    oacc identity-fold run in the tail (halves the DVE traffic of the
    baseline's per-pair copy/add).
  - v-projection evicts batched 2 sk-tiles per copy.
"""

import numpy as np

B, S, D, H = 4, 2048, 512, 8
DH = D // H            # 64
SQL = S // 2           # 1024 queries per core
N_CORES = 8
SK_TILES = S // 128    # 16
VSTRIDE = DH + 1       # per (head, sk-tile) column block in vaug
FAR = 10**6            # deadline for "whenever" chunks

# Schraudolph-on-DVE exp constants (bf16 bit trick, floor conversion)
A16 = 128.0 / np.log(2.0)
B16 = 127.0 * 128.0 - 6.9875

_CACHE = {}
SP_ORDER = [("q", 0), ("q", 1), ("kv", 0, 0), ("kv", 0, 1), ("kv", 1, 0),
            ("kv", 0, 2), ("kv", 1, 1), ("kv", 0, 3), ("kv", 1, 2),
            ("kv", 1, 3)]


def dve_exp_tile(t):
    """Which sk-tiles' exp goes to the DVE (lambda = 4/16)."""
    return False  # BISECT: all ScalarE


def _build_program(repeat=1):
    import concourse.mybir as mybir
    import concourse.tile as tile
    from concourse import bacc
    from collections import deque

    from concourse.masks import make_identity

    F32 = mybir.dt.float32
    BF16 = mybir.dt.bfloat16
    U16 = mybir.dt.uint16
    EXP = mybir.ActivationFunctionType.Exp
    IDENT = mybir.ActivationFunctionType.Identity

    nc = bacc.Bacc(
        "TRN2",
        target_bir_lowering=False,
        debug=False,
        enable_asserts=False,
        num_devices=N_CORES,
    )

    qt_d = nc.dram_tensor("qt", [D, SQL], BF16, kind="ExternalInput").ap()
    kv_d = nc.dram_tensor("kv", [D, 2 * S], BF16, kind="ExternalInput").ap()
    w4_d = nc.dram_tensor("w4", [D, 4 * D], BF16, kind="ExternalInput").ap()
    out_d = nc.dram_tensor("out", [SQL, D], BF16, kind="ExternalOutput").ap()

    with tile.TileContext(nc) as tc:
      for rep_i in range(repeat):
        with (
            tc.tile_pool(name="const", bufs=1) as const_pool,
            tc.tile_pool(name="expt", bufs=34) as exp_pool,
            tc.tile_pool(name="ctxs", bufs=2) as ctxs_pool,
            tc.tile_pool(name="bc", bufs=2) as bc_pool,
            tc.tile_pool(name="small", bufs=2) as small_pool,
            tc.tile_pool(name="outsb", bufs=6) as out_pool,
            tc.tile_pool(name="sc", bufs=1, space="PSUM") as ps_sc,
            tc.tile_pool(name="chunk", bufs=2, space="PSUM") as ps_chunk,
            tc.tile_pool(name="ctx", bufs=1, space="PSUM") as ps_ctx,
        ):
            # ---------- persistent SBUF tensors ----------
            wt = {
                nm: const_pool.tile([128, 4 * D], BF16, name=f"wt_{nm}")
                for nm in ("wq", "wk", "wv", "wo")
            }
            WT = {
                nm: [wt[nm][:, j * D : (j + 1) * D] for j in range(4)]
                for nm in ("wq", "wk", "wv", "wo")
            }
            qtall = const_pool.tile([128, 4 * SQL], BF16, name="qtall")
            QTs = [qtall[:, j * SQL : (j + 1) * SQL] for j in range(4)]
            kvall = const_pool.tile([128, 8 * S], BF16, name="kvall")
            KTs = [kvall[:, j * 2 * S : j * 2 * S + S] for j in range(4)]
            VTs = [kvall[:, j * 2 * S + S : (j + 1) * 2 * S] for j in range(4)]

            qT = [const_pool.tile([128, SQL], BF16, name=f"qT{p}") for p in range(4)]
            kT = [const_pool.tile([128, S], BF16, name=f"kT{p}") for p in range(4)]
            vaug = const_pool.tile([128, H * SK_TILES * VSTRIDE], BF16, name="vaug")
            catT = [
                const_pool.tile([128, SQL], BF16, name=f"catT{p}") for p in range(4)
            ]
            oacc = [
                const_pool.tile([128, D], BF16, name=f"oacc{m}")
                for m in range(SQL // 128)
            ]
            ident_f32 = const_pool.tile([128, 128], F32, name="ident_f32")
            make_identity(nc, ident_f32[:])
            identb = const_pool.tile([128, 128], BF16, name="identb")
            nc.vector.tensor_copy(identb[:], ident_f32[:])
            ones128 = const_pool.tile([128, H * SK_TILES], BF16, name="ones128")
            nc.gpsimd.memset(ones128[:], 1.0)
            vaug4 = vaug[:].rearrange("p (g t e) -> p g t e", g=H, e=VSTRIDE)
            nc.vector.tensor_copy(
                vaug[:].rearrange("p (x e) -> p x e", e=VSTRIDE)[:, :, DH],
                ones128[:],
            )

            # ---------- DMA staging ----------
            # Transfers serialize globally at ~332GB/s; arrival order
            # matches need order (see baseline docstring).
            def dma_pjc(eng, dst_pjc, src_2d):
                eng.dma_start(dst_pjc, src_2d.rearrange("(j p) c -> p j c", j=4))

            for i, nm in enumerate(("wq", "wk", "wv", "wo")):
                dma_pjc(
                    nc.scalar,
                    wt[nm][:].rearrange("p (j c) -> p j c", j=4),
                    w4_d[:, i * D : (i + 1) * D],
                )
            qt3 = qtall[:].rearrange("p (j c) -> p j c", j=4)
            kv4 = kvall[:].rearrange("p (j w b c) -> p j w b c", j=4, w=2, b=4)

            def dma_qt(half):
                cols = slice(half * 512, (half + 1) * 512)
                dma_pjc(nc.sync, qt3[:, :, cols], qt_d[:, cols])

            def dma_kv(w, sb):
                dma_pjc(
                    nc.sync,
                    kv4[:, :, w, sb, :],
                    kv_d[:, w * S + sb * 512 : w * S + (sb + 1) * 512],
                )

            for step in SP_ORDER:
                if step[0] == "q":
                    dma_qt(step[1])
                else:
                    dma_kv(step[1], step[2])

            # ---------- deferred work chunks ----------
            def q_chunk(pr, half):
                cols = slice(half * 512, (half + 1) * 512)
                ps = ps_chunk.tile([128, 512], F32, tag="chunk", name="psq")
                for j in range(4):
                    nc.tensor.matmul(
                        ps[:],
                        WT["wq"][j][:, pr * 128 : (pr + 1) * 128],
                        QTs[j][:, cols],
                        start=(j == 0),
                        stop=(j == 3),
                    )
                nc.vector.tensor_copy(qT[pr][:, cols], ps[:])

            def k_chunk(pr, sb):
                cols = slice(sb * 512, (sb + 1) * 512)
                ps = ps_chunk.tile([128, 512], F32, tag="chunk", name="psk")
                for j in range(4):
                    nc.tensor.matmul(
                        ps[:],
                        WT["wk"][j][:, pr * 128 : (pr + 1) * 128],
                        KTs[j][:, cols],
                        start=(j == 0),
                        stop=(j == 3),
                    )
                nc.vector.tensor_copy(kT[pr][:, cols], ps[:])

            def v_chunk2(pr, sg):
                # two sk-tiles (st=2*sg, 2*sg+1) per psum tile: one evict
                ps = ps_chunk.tile([128, 256], F32, tag="chunk", name="psv")
                for half in range(2):
                    st = 2 * sg + half
                    for j in range(4):
                        nc.tensor.matmul(
                            ps[:, half * 128 : (half + 1) * 128],
                            VTs[j][:, st * 128 : (st + 1) * 128],
                            WT["wv"][j][:, pr * 128 : (pr + 1) * 128],
                            start=(j == 0),
                            stop=(j == 3),
                        )
                nc.vector.tensor_copy(
                    vaug4[:, 2 * pr : 2 * pr + 2, 2 * sg : 2 * sg + 2, 0:DH],
                    ps[:].rearrange("p (t g e) -> p g t e", t=2, g=2),
                )

            def o2_chunk(m):
                # pairs 0+1 output-projection partials folded in one psum
                # accumulation group -> single DVE copy
                ps = ps_chunk.tile([128, 512], F32, tag="chunk", name="pso2")
                nc.tensor.matmul(
                    ps[:],
                    catT[0][:, m * 128 : (m + 1) * 128],
                    WT["wo"][0][:],
                    start=True,
                    stop=False,
                )
                nc.tensor.matmul(
                    ps[:],
                    catT[1][:, m * 128 : (m + 1) * 128],
                    WT["wo"][1][:],
                    start=False,
                    stop=True,
                )
                nc.vector.tensor_copy(oacc[m][:], ps[:])

            def o1_chunk(m):
                # pair 2 partial: one DVE add
                ps = ps_chunk.tile([128, 512], F32, tag="chunk", name="pso1")
                nc.tensor.matmul(
                    ps[:],
                    catT[2][:, m * 128 : (m + 1) * 128],
                    WT["wo"][2][:],
                    start=True,
                    stop=True,
                )
                nc.vector.tensor_add(oacc[m][:], oacc[m][:], ps[:])

            def pair_chunks(pr, skip_prefix=False):
                """(deadline, closure); deadline in global slot index
                g = pr*16 + t."""
                g0 = 16 * pr
                out = []
                if not skip_prefix:
                    out.append((g0 - 3, lambda pr=pr: q_chunk(pr, 0)))
                    out.append((g0 - 3, lambda pr=pr: q_chunk(pr, 1)))
                    out.append((g0 - 3, lambda pr=pr: k_chunk(pr, 0)))
                for sb in range(1, 4):
                    out.append((g0 + 4 * sb - 1, lambda pr=pr, sb=sb: k_chunk(pr, sb)))
                # ctx half0 of (pr, t) issues at slot pr*16 + 8 + t//2
                for sg in range(8):
                    out.append(
                        (g0 + 7 + sg, lambda pr=pr, sg=sg: v_chunk2(pr, sg))
                    )
                out.sort(key=lambda c: c[0])
                return out

            queue = deque()
            # pair 0's q/k head-start runs before slot 0 (prefix)
            q_chunk(0, 0)
            q_chunk(0, 1)
            k_chunk(0, 0)
            queue.extend(pair_chunks(0, skip_prefix=True))
            for pr in range(1, 4):
                queue.extend(pair_chunks(pr))
            queue = deque(sorted(queue, key=lambda c: c[0]))

            def service(g, relaxed=False):
                pulled = 0
                while queue:
                    viol = any(d <= g + i for i, (d, _) in enumerate(queue))
                    backlog = len(queue) > (62 - g) and pulled < 3
                    if viol or ((pulled == 0 or backlog) and not relaxed):
                        _, fn = queue.popleft()
                        fn()
                        pulled += 1
                    else:
                        break

            # ---------- attention: paired heads, split-half ctx ----------
            def issue_sc_pair(pr, t):
                """Row-tiled concurrent scores for heads a=2pr (rows 0:64)
                and b=2pr+1 (rows 64:128)."""
                sca = ps_sc.tile([128, SQL], F32, tag="sca", name="sca")
                scb = ps_sc.tile([128, SQL], F32, tag="scb", name="scb")
                for c in range(2):
                    for a, sc in ((0, sca), (1, scb)):
                        rows = slice(a * DH, (a + 1) * DH)
                        nc.tensor.matmul(
                            sc[:, c * 512 : (c + 1) * 512],
                            kT[pr][rows, t * 128 : (t + 1) * 128],
                            qT[pr][rows, c * 512 : (c + 1) * 512],
                            start=True,
                            stop=True,
                        )
                return sca, scb

            def issue_exp(h, t, sc):
                et = exp_pool.tile([128, SQL], BF16, tag="expt", name="expt")
                if dve_exp_tile(t):
                    nc.vector.tensor_scalar(
                        et[:].bitcast(U16), sc[:],
                        float(A16 * 0.125), float(B16),
                        mybir.AluOpType.mult, mybir.AluOpType.add,
                    )
                else:
                    nc.scalar.activation(et[:], sc[:], EXP, scale=0.125)
                return et

            def ctx_mm(h, t, ctx, half, start, stop):
                pr = h // 2
                c0 = (h * SK_TILES + t) * VSTRIDE
                et = ets[(h, t)]
                nc.tensor.matmul(
                    ctx[:],
                    vaug[:, c0 : c0 + VSTRIDE],
                    et[:, half * 512 : (half + 1) * 512],
                    start=start,
                    stop=stop,
                )

            def norm_prep(src, src_cols, n):
                """sums -> reciprocal -> partition-broadcast; src is a
                [65, n-cols] view (psum or sbuf)."""
                sums = small_pool.tile([1, 512], F32, tag="sums", name="sums")
                nc.vector.tensor_copy(sums[0:1, 0:n], src[DH : DH + 1, src_cols])
                recip = small_pool.tile([1, 512], F32, tag="recip", name="recip")
                nc.vector.reciprocal_approx_fast(recip[0:1, 0:n], sums[0:1, 0:n])
                bc = bc_pool.tile([DH, 512], F32, tag="bc", name="bc")
                nc.gpsimd.partition_broadcast(bc[:, 0:n], recip[0:1, 0:n])
                return bc

            def normalize_half(h, ctx_sb, half):
                """ctx_sb: [65, 512] SBUF tile holding the accumulated
                (ctx|sums) for query half `half` of head h."""
                pr, a = h // 2, h % 2
                rows = slice(a * DH, (a + 1) * DH)
                cols = slice(half * 512, (half + 1) * 512)
                bc = norm_prep(ctx_sb, slice(0, 512), 512)
                nc.gpsimd.tensor_mul(
                    catT[pr][rows, cols], ctx_sb[0:DH, 0:512], bc[:, 0:512]
                )

            ets = {}
            ctx_tiles = {}

            def evict_norm_half0(h):
                ctx = ctx_tiles[(h, 0)]
                ctxs = ctxs_pool.tile([DH + 1, 512], F32, tag="ctxs", name="ctxs")
                nc.vector.tensor_copy(ctxs[:], ctx[:])
                normalize_half(h, ctxs, 0)

            # slot structure per pair pr (g = pr*16 + t):
            #   slots 0-7 : prev pair's ctx half1 (4 matmuls/slot)
            #   all slots : sc pair (t), exp pair (t)
            #   slots 8-15: this pair's ctx half0 (4 matmuls/slot)
            for pr in range(4):
                a, b = 2 * pr, 2 * pr + 1
                prev = pr - 1
                for t in range(SK_TILES):
                    g = pr * 16 + t
                    sca, scb = issue_sc_pair(pr, t)
                    ets[(a, t)] = issue_exp(a, t, sca)
                    ets[(b, t)] = issue_exp(b, t, scb)
                    # prev pair's ctx half1: 4 matmuls per slot 0-7
                    if prev >= 0 and t < 8:
                        if t == 0:
                            ctx_tiles[(2 * prev, 1)] = ps_ctx.tile(
                                [DH + 1, 512], F32, tag="ca", name="ca1"
                            )
                            ctx_tiles[(2 * prev + 1, 1)] = ps_ctx.tile(
                                [DH + 1, 512], F32, tag="cb", name="cb1"
                            )
                        for hh in (2 * prev, 2 * prev + 1):
                            for tpos in (2 * t, 2 * t + 1):
                                ctx_mm(
                                    hh, tpos, ctx_tiles[(hh, 1)], 1,
                                    start=(tpos == 0), stop=(tpos == 15),
                                )
                        if t == 7:
                            for hh in (2 * prev, 2 * prev + 1):
                                ctx = ctx_tiles[(hh, 1)]
                                ctxs = ctxs_pool.tile(
                                    [DH + 1, 512], F32, tag="ctxs", name="ctxs"
                                )
                                nc.vector.tensor_copy(ctxs[:], ctx[:])
                                normalize_half(hh, ctxs, 1)
                                for kk, tt in list(ets.keys()):
                                    if kk == hh:
                                        del ets[(kk, tt)]
                            if prev == 1:
                                # catT 0+1 fully ready: fold their
                                # output-projection partials
                                for m in range(8):
                                    queue.append((pr * 16 + 8 + m // 2,
                                                  lambda m=m: o2_chunk(m)))
                            if prev == 2:
                                for m in range(8):
                                    queue.append((pr * 16 + 8 + m // 2,
                                                  lambda m=m: o1_chunk(m)))
                    # this pair's ctx half0: 4 matmuls per slot 8-15
                    if t >= 8:
                        if t == 8:
                            ctx_tiles[(a, 0)] = ps_ctx.tile(
                                [DH + 1, 512], F32, tag="ca", name="ca0"
                            )
                            ctx_tiles[(b, 0)] = ps_ctx.tile(
                                [DH + 1, 512], F32, tag="cb", name="cb0"
                            )
                        for hh in (a, b):
                            for tpos in (2 * (t - 8), 2 * (t - 8) + 1):
                                ctx_mm(
                                    hh, tpos, ctx_tiles[(hh, 0)], 0,
                                    start=(tpos == 0), stop=(tpos == 15),
                                )
                    service(g, relaxed=(t >= 14))
                # pair boundary: evict + normalize half0 (frees ctx psum
                # for this pair's half1 in next pair's slots)
                evict_norm_half0(a)
                evict_norm_half0(b)

            # ---------- tail: pair 3 ctx half1 + final projection ----------
            pr = 3
            a, b = 6, 7
            ctx_tiles[(a, 1)] = ps_ctx.tile([DH + 1, 512], F32, tag="ca", name="ca1")
            ctx_tiles[(b, 1)] = ps_ctx.tile([DH + 1, 512], F32, tag="cb", name="cb1")
            for hh in (a, b):
                for t in range(SK_TILES):
                    ctx_mm(hh, t, ctx_tiles[(hh, 1)], 1,
                           start=(t == 0), stop=(t == 15))
            while queue:
                _, fn = queue.popleft()
                fn()

            # last pair half1: normalize straight from PSUM in half-column
            # chains pipelined across DVE/gpsimd
            for hh in (a, b):
                ctx = ctx_tiles[(hh, 1)]
                rows = slice((hh % 2) * DH, (hh % 2 + 1) * DH)
                h2 = [slice(0, 256), slice(256, 512)]
                cat_h2 = [slice(512, 768), slice(768, 1024)]
                bcs = []
                bcs.append(norm_prep(ctx, h2[0], 256))
                bcs.append(norm_prep(ctx, h2[1], 256))
                nc.vector.tensor_mul(
                    catT[3][rows, cat_h2[0]], ctx[0:DH, h2[0]], bcs[0][:, 0:256]
                )
                nc.vector.tensor_mul(
                    catT[3][rows, cat_h2[1]], ctx[0:DH, h2[1]], bcs[1][:, 0:256]
                )

            # final output projection: pair 3 matmul + identity-fold of
            # oacc, ScalarE/DVE alternate evictions, two DMA queues
            def emit_ident(m):
                pool, tag = (
                    (ps_chunk, "chunk") if m % 2 == 0 else (ps_sc, "sca")
                )
                ps = pool.tile([128, 512], F32, tag=tag, name="pso3")
                nc.tensor.matmul(
                    ps[:], identb[:], oacc[m][:], start=True, stop=False
                )
                return ps

            pss = [emit_ident(m) for m in range(4)]
            for m in range(SQL // 128):
                ps = pss[m]
                nc.tensor.matmul(
                    ps[:],
                    catT[3][:, m * 128 : (m + 1) * 128],
                    WT["wo"][3][:],
                    start=False,
                    stop=True,
                )
                ot = out_pool.tile([128, D], BF16, tag="ot", name="ot")
                if m % 2 == 0:
                    nc.vector.tensor_copy(ot[:], ps[:])
                else:
                    nc.scalar.activation(ot[:], ps[:], IDENT)
                deng = nc.sync if m % 2 == 0 else nc.scalar
                deng.dma_start(out_d[m * 128 : (m + 1) * 128, :], ot[:])
                if m + 4 < SQL // 128:
                    pss.append(emit_ident(m + 4))

    nc.compile()
    return nc


def _get_nc():
    if "nc" not in _CACHE:
        _CACHE["nc"] = _build_program()
    return _CACHE["nc"]


def make_in_maps(Q, K, V, Wq, Wk, Wv, Wout):
    import ml_dtypes

    BF = ml_dtypes.bfloat16

    def t(x):  # [r, c] fp32-ish -> bf16 [c, r]
        return np.asarray(x, dtype=np.float32).T.astype(BF)

    w4 = np.ascontiguousarray(
        np.concatenate(
            [
                t(np.asarray(w, dtype=np.float32).reshape(D, D))
                for w in (Wq, Wk, Wv, Wout)
            ],
            axis=1,
        )
    )
    Q = np.asarray(Q, dtype=np.float32)
    K = np.asarray(K, dtype=np.float32)
    V = np.asarray(V, dtype=np.float32)
    kv = [
        np.ascontiguousarray(np.concatenate([t(K[b]), t(V[b])], axis=1))
        for b in range(B)
    ]
    in_maps = []
    for c in range(N_CORES):
        b, qh = c // 2, c % 2
        in_maps.append(
            {
                "qt": np.ascontiguousarray(t(Q[b, qh * SQL : (qh + 1) * SQL, :])),
                "kv": kv[b],
                "w4": w4,
            }
        )
    return in_maps


def assemble_out(results):
    out = np.empty((B, S, D), dtype=np.float32)
    for c in range(N_CORES):
        b, qh = c // 2, c % 2
        out[b, qh * SQL : (qh + 1) * SQL, :] = np.asarray(
            results[c]["out"]
        ).astype(np.float32)
    return out


def kernel(Q, K, V, mask=None, Wq=None, Wk=None, Wv=None, Wout=None):
    # mask is a per-query additive constant before softmax -> softmax is
    # invariant to it; skipping it is numerically exact.
    from concourse.bass_utils import run_bass_kernel_spmd

    nc = _get_nc()
    in_maps = make_in_maps(Q, K, V, Wq, Wk, Wv, Wout)
    res = run_bass_kernel_spmd(nc, in_maps, core_ids=list(range(N_CORES)))
    return assemble_out(res.results)


if __name__ == "__main__":
    rng = np.random.default_rng(0)
    ins = {
        "Q": rng.standard_normal((B, S, D), dtype=np.float32),
        "K": rng.standard_normal((B, S, D), dtype=np.float32),
        "V": rng.standard_normal((B, S, D), dtype=np.float32),
        "mask": np.zeros((B, S), np.int32),
        "Wq": rng.standard_normal((H, DH, D), dtype=np.float32) / np.sqrt(D),
        "Wk": rng.standard_normal((H, DH, D), dtype=np.float32) / np.sqrt(D),
        "Wv": rng.standard_normal((H, DH, D), dtype=np.float32) / np.sqrt(D),
        "Wout": rng.standard_normal((D, D), dtype=np.float32) / np.sqrt(D),
    }
    out = kernel(**ins)
    print("out", out.shape, out.dtype, float(np.abs(out).max()))


# revision 7
# speedup vs baseline: 1.3673x; 1.3673x over previous
"""Multi-head attention Trainium2 Bass kernel (8 NeuronCores, SPMD).

Problem: B=4, S=2048, D=512, H=8 heads of DH=64.
  q = Q @ Wq[h].T ; k = K @ Wk[h].T ; v = V @ Wv[h].T     (per head)
  scores = q @ k.T / sqrt(DH)   (+ mask term: a per-query constant,
           which softmax is invariant to -> ignored)
  attn = softmax(scores, axis=keys)
  out  = concat_h(attn @ v) @ Wout.T

Sharding: core c handles batch b=c//2, query half qh=c%2 -> each core
computes a [1024, 512] slice of the output independently (no
collectives).

v2 design (baseline measured 188us; PE-busy model ~125us):
  - All matmul operands bf16; PSUM accumulation f32.
  - Scores matmuls (K=DH=64, 50% PE util) run as ROW-TILED HEAD PAIRS:
    head a of pair pr lives on partitions 0-63 of qT/kT, head b on
    64-127; consecutive matmuls at tile_position (0,0)/(64,0) execute
    CONCURRENTLY on disjoint row-groups of the PE array (HW-measured
    2.04x vs sequential) -> scores PE time halves.
  - Softmax exp split across engines: ScalarE computes exact exp for
    3/4 of (h,t) score tiles; the otherwise-idle DVE computes the
    remaining 1/4 via a Schraudolph bit-trick (one tensor_scalar:
    u16 = floor(scores*(128*log2e/8) + (127*128 - 6.9875)), bitcast
    bf16 ~= exp(scores/8), rel err ~1.8% RMS on those tiles; end-to-end
    rel err measured 1.2e-2 in numpy pipeline sim, threshold 2e-2).
  - ctx (attn @ v, M=65 with appended ones column accumulating softmax
    denominators) is split by query half: half0 [65,512] accumulates
    during the pair's slots 8-15 (4/slot), half1 re-reads the retained
    et tiles during the NEXT pair's slots 0-7.  This lets sc psum
    (2x[128,1024]) + ctx psum (2x[65,512]) + chunk psum (2x[128,512])
    fit exactly in the 8 PSUM banks with no double-buffer stalls.
  - Deferred chunk queue (q/k/v projections, output-projection
    partials) interleaves into the per-slot stream as PE filler, as in
    the baseline.
  - Output projection: partials of pairs 0+1 fold in one psum
    accumulation group (single DVE copy), pair 2 adds once, pair 3 +
    oacc identity-fold run in the tail (halves the DVE traffic of the
    baseline's per-pair copy/add).
  - v-projection evicts batched 2 sk-tiles per copy.
"""

import numpy as np

B, S, D, H = 4, 2048, 512, 8
DH = D // H            # 64
SQL = S // 2           # 1024 queries per core
N_CORES = 8
SK_TILES = S // 128    # 16
VSTRIDE = DH + 1       # per (head, sk-tile) column block in vaug
FAR = 10**6            # deadline for "whenever" chunks

# Schraudolph-on-DVE exp constants (bf16 bit trick, floor conversion)
A16 = 128.0 / np.log(2.0)
B16 = 127.0 * 128.0 - 6.9875

_CACHE = {}
import os as _os
BISECT = set(filter(None, _os.environ.get("BISECT", "").split(",")))
SP_ORDER = [("q", 0), ("q", 1), ("kv", 0, 0), ("kv", 0, 1), ("kv", 1, 0),
            ("kv", 0, 2), ("kv", 1, 1), ("kv", 0, 3), ("kv", 1, 2),
            ("kv", 1, 3)]


def dve_exp_tile(t):
    """Which sk-tiles' exp goes to the DVE (lambda = 4/16)."""
    return t % 4 == 3


def _build_program(repeat=1):
    import concourse.mybir as mybir
    import concourse.tile as tile
    from concourse import bacc
    from collections import deque

    from concourse.masks import make_identity

    F32 = mybir.dt.float32
    BF16 = mybir.dt.bfloat16
    U16 = mybir.dt.uint16
    EXP = mybir.ActivationFunctionType.Exp
    IDENT = mybir.ActivationFunctionType.Identity

    nc = bacc.Bacc(
        "TRN2",
        target_bir_lowering=False,
        debug=False,
        enable_asserts=False,
        num_devices=N_CORES,
    )

    qt_d = nc.dram_tensor("qt", [D, SQL], BF16, kind="ExternalInput").ap()
    kv_d = nc.dram_tensor("kv", [D, 2 * S], BF16, kind="ExternalInput").ap()
    w4_d = nc.dram_tensor("w4", [D, 4 * D], BF16, kind="ExternalInput").ap()
    out_d = nc.dram_tensor("out", [SQL, D], BF16, kind="ExternalOutput").ap()

    with tile.TileContext(nc) as tc:
      for rep_i in range(repeat):
        with (
            tc.tile_pool(name="const", bufs=1) as const_pool,
            tc.tile_pool(name="expt", bufs=34) as exp_pool,
            tc.tile_pool(name="ctxs", bufs=2) as ctxs_pool,
            tc.tile_pool(name="bc", bufs=2) as bc_pool,
            tc.tile_pool(name="small", bufs=2) as small_pool,
            tc.tile_pool(name="outsb", bufs=6) as out_pool,
            tc.tile_pool(name="sc", bufs=1, space="PSUM") as ps_sc,
            tc.tile_pool(name="chunk", bufs=2, space="PSUM") as ps_chunk,
            tc.tile_pool(name="ctx", bufs=1, space="PSUM") as ps_ctx,
        ):
            # ---------- persistent SBUF tensors ----------
            wt = {
                nm: const_pool.tile([128, 4 * D], BF16, name=f"wt_{nm}")
                for nm in ("wq", "wk", "wv", "wo")
            }
            WT = {
                nm: [wt[nm][:, j * D : (j + 1) * D] for j in range(4)]
                for nm in ("wq", "wk", "wv", "wo")
            }
            qtall = const_pool.tile([128, 4 * SQL], BF16, name="qtall")
            QTs = [qtall[:, j * SQL : (j + 1) * SQL] for j in range(4)]
            kvall = const_pool.tile([128, 8 * S], BF16, name="kvall")
            KTs = [kvall[:, j * 2 * S : j * 2 * S + S] for j in range(4)]
            VTs = [kvall[:, j * 2 * S + S : (j + 1) * 2 * S] for j in range(4)]

            qT = [const_pool.tile([128, SQL], BF16, name=f"qT{p}") for p in range(4)]
            kT = [const_pool.tile([128, S], BF16, name=f"kT{p}") for p in range(4)]
            vaug = const_pool.tile([128, H * SK_TILES * VSTRIDE], BF16, name="vaug")
            catT = [
                const_pool.tile([128, SQL], BF16, name=f"catT{p}") for p in range(4)
            ]
            oacc = [
                const_pool.tile([128, D], BF16, name=f"oacc{m}")
                for m in range(SQL // 128)
            ]
            ident_f32 = const_pool.tile([128, 128], F32, name="ident_f32")
            make_identity(nc, ident_f32[:])
            identb = const_pool.tile([128, 128], BF16, name="identb")
            nc.vector.tensor_copy(identb[:], ident_f32[:])
            ones128 = const_pool.tile([128, H * SK_TILES], BF16, name="ones128")
            nc.gpsimd.memset(ones128[:], 1.0)
            vaug4 = vaug[:].rearrange("p (g t e) -> p g t e", g=H, e=VSTRIDE)
            nc.vector.tensor_copy(
                vaug[:].rearrange("p (x e) -> p x e", e=VSTRIDE)[:, :, DH],
                ones128[:],
            )

            # ---------- DMA staging ----------
            # Transfers serialize globally at ~332GB/s; arrival order
            # matches need order (see baseline docstring).
            def dma_pjc(eng, dst_pjc, src_2d):
                eng.dma_start(dst_pjc, src_2d.rearrange("(j p) c -> p j c", j=4))

            for i, nm in enumerate(("wq", "wk", "wv", "wo")):
                dma_pjc(
                    nc.scalar,
                    wt[nm][:].rearrange("p (j c) -> p j c", j=4),
                    w4_d[:, i * D : (i + 1) * D],
                )
            qt3 = qtall[:].rearrange("p (j c) -> p j c", j=4)
            kv4 = kvall[:].rearrange("p (j w b c) -> p j w b c", j=4, w=2, b=4)

            def dma_qt(half):
                cols = slice(half * 512, (half + 1) * 512)
                dma_pjc(nc.sync, qt3[:, :, cols], qt_d[:, cols])

            def dma_kv(w, sb):
                dma_pjc(
                    nc.sync,
                    kv4[:, :, w, sb, :],
                    kv_d[:, w * S + sb * 512 : w * S + (sb + 1) * 512],
                )

            for step in SP_ORDER:
                if step[0] == "q":
                    dma_qt(step[1])
                else:
                    dma_kv(step[1], step[2])

            # ---------- deferred work chunks ----------
            def q_chunk(pr, half):
                cols = slice(half * 512, (half + 1) * 512)
                ps = ps_chunk.tile([128, 512], F32, tag="chunk", name="psq")
                for j in range(4):
                    nc.tensor.matmul(
                        ps[:],
                        WT["wq"][j][:, pr * 128 : (pr + 1) * 128],
                        QTs[j][:, cols],
                        start=(j == 0),
                        stop=(j == 3),
                    )
                nc.vector.tensor_copy(qT[pr][:, cols], ps[:])

            def k_chunk(pr, sb):
                cols = slice(sb * 512, (sb + 1) * 512)
                ps = ps_chunk.tile([128, 512], F32, tag="chunk", name="psk")
                for j in range(4):
                    nc.tensor.matmul(
                        ps[:],
                        WT["wk"][j][:, pr * 128 : (pr + 1) * 128],
                        KTs[j][:, cols],
                        start=(j == 0),
                        stop=(j == 3),
                    )
                nc.vector.tensor_copy(kT[pr][:, cols], ps[:])

            def v_chunk2(pr, sg):
                # two sk-tiles (st=2*sg, 2*sg+1) per psum tile: one evict
                ps = ps_chunk.tile([128, 256], F32, tag="chunk", name="psv")
                for half in range(2):
                    st = 2 * sg + half
                    for j in range(4):
                        nc.tensor.matmul(
                            ps[:, half * 128 : (half + 1) * 128],
                            VTs[j][:, st * 128 : (st + 1) * 128],
                            WT["wv"][j][:, pr * 128 : (pr + 1) * 128],
                            start=(j == 0),
                            stop=(j == 3),
                        )
                nc.vector.tensor_copy(
                    vaug4[:, 2 * pr : 2 * pr + 2, 2 * sg : 2 * sg + 2, 0:DH],
                    ps[:].rearrange("p (t g e) -> p g t e", t=2, g=2),
                )

            def o2_chunk(m):
                # pairs 0+1 output-projection partials folded in one psum
                # accumulation group -> single DVE copy
                ps = ps_chunk.tile([128, 512], F32, tag="chunk", name="pso2")
                nc.tensor.matmul(
                    ps[:],
                    catT[0][:, m * 128 : (m + 1) * 128],
                    WT["wo"][0][:],
                    start=True,
                    stop=False,
                )
                nc.tensor.matmul(
                    ps[:],
                    catT[1][:, m * 128 : (m + 1) * 128],
                    WT["wo"][1][:],
                    start=False,
                    stop=True,
                )
                nc.vector.tensor_copy(oacc[m][:], ps[:])

            def o1_chunk(m):
                # pair 2 partial: one DVE add
                ps = ps_chunk.tile([128, 512], F32, tag="chunk", name="pso1")
                nc.tensor.matmul(
                    ps[:],
                    catT[2][:, m * 128 : (m + 1) * 128],
                    WT["wo"][2][:],
                    start=True,
                    stop=True,
                )
                nc.vector.tensor_add(oacc[m][:], oacc[m][:], ps[:])

            def pair_chunks(pr, skip_prefix=False):
                """(deadline, closure); deadline in global slot index
                g = pr*16 + t."""
                g0 = 16 * pr
                out = []
                if not skip_prefix:
                    out.append((g0 - 3, lambda pr=pr: q_chunk(pr, 0)))
                    out.append((g0 - 3, lambda pr=pr: q_chunk(pr, 1)))
                    out.append((g0 - 3, lambda pr=pr: k_chunk(pr, 0)))
                for sb in range(1, 4):
                    out.append((g0 + 4 * sb - 1, lambda pr=pr, sb=sb: k_chunk(pr, sb)))
                # ctx half0 of (pr, t) issues at slot pr*16 + 8 + t//2
                for sg in range(8):
                    out.append(
                        (g0 + 7 + sg, lambda pr=pr, sg=sg: v_chunk2(pr, sg))
                    )
                out.sort(key=lambda c: c[0])
                return out

            queue = deque()
            # pair 0's q/k head-start runs before slot 0 (prefix)
            q_chunk(0, 0)
            q_chunk(0, 1)
            k_chunk(0, 0)
            queue.extend(pair_chunks(0, skip_prefix=True))
            for pr in range(1, 4):
                queue.extend(pair_chunks(pr))
            queue = deque(sorted(queue, key=lambda c: c[0]))

            def service(g, relaxed=False):
                pulled = 0
                while queue:
                    viol = any(d <= g + i for i, (d, _) in enumerate(queue))
                    backlog = len(queue) > (62 - g) and pulled < 3
                    if viol or ((pulled == 0 or backlog) and not relaxed):
                        _, fn = queue.popleft()
                        fn()
                        pulled += 1
                    else:
                        break

            # ---------- attention: paired heads, split-half ctx ----------
            def issue_sc_pair(pr, t):
                """Row-tiled concurrent scores for heads a=2pr (rows 0:64)
                and b=2pr+1 (rows 64:128)."""
                sca = ps_sc.tile([128, SQL], F32, tag="sca", name="sca")
                scb = ps_sc.tile([128, SQL], F32, tag="scb", name="scb")
                w = 64 if "nosc" in BISECT else 512
                for c in range(2):
                    for a, sc in ((0, sca), (1, scb)):
                        rows = slice(a * DH, (a + 1) * DH)
                        nc.tensor.matmul(
                            sc[:, c * 512 : c * 512 + w],
                            kT[pr][rows, t * 128 : (t + 1) * 128],
                            qT[pr][rows, c * 512 : c * 512 + w],
                            start=True,
                            stop=True,
                        )
                return sca, scb

            def issue_exp(h, t, sc):
                et = exp_pool.tile([128, SQL], BF16, tag="expt", name="expt")
                ecols = slice(0, 64) if "noexp" in BISECT else slice(0, SQL)
                if dve_exp_tile(t):
                    nc.vector.tensor_scalar(
                        et[:, ecols].bitcast(U16), sc[:, ecols],
                        float(A16 * 0.125), float(B16),
                        mybir.AluOpType.mult, mybir.AluOpType.add,
                    )
                else:
                    nc.scalar.activation(et[:, ecols], sc[:, ecols], EXP, scale=0.125)
                return et

            def ctx_mm(h, t, ctx, half, start, stop):
                pr = h // 2
                c0 = (h * SK_TILES + t) * VSTRIDE
                et = ets[(h, t)]
                w = 64 if "noctx" in BISECT else 512
                nc.tensor.matmul(
                    ctx[:, 0:w],
                    vaug[:, c0 : c0 + VSTRIDE],
                    et[:, half * 512 : half * 512 + w],
                    start=start,
                    stop=stop,
                )

            def norm_prep(src, src_cols, n):
                """sums -> reciprocal -> partition-broadcast; src is a
                [65, n-cols] view (psum or sbuf)."""
                sums = small_pool.tile([1, 512], F32, tag="sums", name="sums")
                nc.vector.tensor_copy(sums[0:1, 0:n], src[DH : DH + 1, src_cols])
                recip = small_pool.tile([1, 512], F32, tag="recip", name="recip")
                nc.vector.reciprocal_approx_fast(recip[0:1, 0:n], sums[0:1, 0:n])
                bc = bc_pool.tile([DH, 512], F32, tag="bc", name="bc")
                nc.gpsimd.partition_broadcast(bc[:, 0:n], recip[0:1, 0:n])
                return bc

            def normalize_half(h, ctx_sb, half):
                """ctx_sb: [65, 512] SBUF tile holding the accumulated
                (ctx|sums) for query half `half` of head h."""
                pr, a = h // 2, h % 2
                rows = slice(a * DH, (a + 1) * DH)
                cols = slice(half * 512, (half + 1) * 512)
                bc = norm_prep(ctx_sb, slice(0, 512), 512)
                nc.gpsimd.tensor_mul(
                    catT[pr][rows, cols], ctx_sb[0:DH, 0:512], bc[:, 0:512]
                )

            ets = {}
            ctx_tiles = {}

            def evict_norm_half0(h):
                ctx = ctx_tiles[(h, 0)]
                ctxs = ctxs_pool.tile([DH + 1, 512], F32, tag="ctxs", name="ctxs")
                nc.vector.tensor_copy(ctxs[:], ctx[:])
                normalize_half(h, ctxs, 0)

            # slot structure per pair pr (g = pr*16 + t):
            #   slots 0-7 : prev pair's ctx half1 (4 matmuls/slot)
            #   all slots : sc pair (t), exp pair (t)
            #   slots 8-15: this pair's ctx half0 (4 matmuls/slot)
            for pr in range(4):
                a, b = 2 * pr, 2 * pr + 1
                prev = pr - 1
                for t in range(SK_TILES):
                    g = pr * 16 + t
                    sca, scb = issue_sc_pair(pr, t)
                    ets[(a, t)] = issue_exp(a, t, sca)
                    ets[(b, t)] = issue_exp(b, t, scb)
                    # prev pair's ctx half1: 4 matmuls per slot 0-7
                    if prev >= 0 and t < 8:
                        if t == 0:
                            ctx_tiles[(2 * prev, 1)] = ps_ctx.tile(
                                [DH + 1, 512], F32, tag="ca", name="ca1"
                            )
                            ctx_tiles[(2 * prev + 1, 1)] = ps_ctx.tile(
                                [DH + 1, 512], F32, tag="cb", name="cb1"
                            )
                        for hh in (2 * prev, 2 * prev + 1):
                            for tpos in (2 * t, 2 * t + 1):
                                ctx_mm(
                                    hh, tpos, ctx_tiles[(hh, 1)], 1,
                                    start=(tpos == 0), stop=(tpos == 15),
                                )
                        if t == 7:
                            for hh in (2 * prev, 2 * prev + 1):
                                ctx = ctx_tiles[(hh, 1)]
                                ctxs = ctxs_pool.tile(
                                    [DH + 1, 512], F32, tag="ctxs", name="ctxs"
                                )
                                nc.vector.tensor_copy(ctxs[:], ctx[:])
                                normalize_half(hh, ctxs, 1)
                                for kk, tt in list(ets.keys()):
                                    if kk == hh:
                                        del ets[(kk, tt)]
                            if prev == 1:
                                # catT 0+1 fully ready: fold their
                                # output-projection partials
                                for m in range(8):
                                    queue.append((pr * 16 + 8 + m // 2,
                                                  lambda m=m: o2_chunk(m)))
                            if prev == 2:
                                for m in range(8):
                                    queue.append((pr * 16 + 8 + m // 2,
                                                  lambda m=m: o1_chunk(m)))
                    # this pair's ctx half0: 4 matmuls per slot 8-15
                    if t >= 8:
                        if t == 8:
                            ctx_tiles[(a, 0)] = ps_ctx.tile(
                                [DH + 1, 512], F32, tag="ca", name="ca0"
                            )
                            ctx_tiles[(b, 0)] = ps_ctx.tile(
                                [DH + 1, 512], F32, tag="cb", name="cb0"
                            )
                        for hh in (a, b):
                            for tpos in (2 * (t - 8), 2 * (t - 8) + 1):
                                ctx_mm(
                                    hh, tpos, ctx_tiles[(hh, 0)], 0,
                                    start=(tpos == 0), stop=(tpos == 15),
                                )
                    service(g, relaxed=(t >= 14))
                # pair boundary: evict + normalize half0 (frees ctx psum
                # for this pair's half1 in next pair's slots)
                evict_norm_half0(a)
                evict_norm_half0(b)

            # ---------- tail: pair 3 ctx half1 + final projection ----------
            pr = 3
            a, b = 6, 7
            ctx_tiles[(a, 1)] = ps_ctx.tile([DH + 1, 512], F32, tag="ca", name="ca1")
            ctx_tiles[(b, 1)] = ps_ctx.tile([DH + 1, 512], F32, tag="cb", name="cb1")
            for hh in (a, b):
                for t in range(SK_TILES):
                    ctx_mm(hh, t, ctx_tiles[(hh, 1)], 1,
                           start=(t == 0), stop=(t == 15))
            while queue:
                _, fn = queue.popleft()
                fn()

            # last pair half1: normalize straight from PSUM in half-column
            # chains pipelined across DVE/gpsimd
            for hh in (a, b):
                ctx = ctx_tiles[(hh, 1)]
                rows = slice((hh % 2) * DH, (hh % 2 + 1) * DH)
                h2 = [slice(0, 256), slice(256, 512)]
                cat_h2 = [slice(512, 768), slice(768, 1024)]
                bcs = []
                bcs.append(norm_prep(ctx, h2[0], 256))
                bcs.append(norm_prep(ctx, h2[1], 256))
                nc.vector.tensor_mul(
                    catT[3][rows, cat_h2[0]], ctx[0:DH, h2[0]], bcs[0][:, 0:256]
                )
                nc.vector.tensor_mul(
                    catT[3][rows, cat_h2[1]], ctx[0:DH, h2[1]], bcs[1][:, 0:256]
                )

            # final output projection: pair 3 matmul + identity-fold of
            # oacc, ScalarE/DVE alternate evictions, two DMA queues
            def emit_ident(m):
                pool, tag = (
                    (ps_chunk, "chunk") if m % 2 == 0 else (ps_sc, "sca")
                )
                ps = pool.tile([128, 512], F32, tag=tag, name="pso3")
                nc.tensor.matmul(
                    ps[:], identb[:], oacc[m][:], start=True, stop=False
                )
                return ps

            pss = [emit_ident(m) for m in range(4)]
            for m in range(SQL // 128):
                ps = pss[m]
                nc.tensor.matmul(
                    ps[:],
                    catT[3][:, m * 128 : (m + 1) * 128],
                    WT["wo"][3][:],
                    start=False,
                    stop=True,
                )
                ot = out_pool.tile([128, D], BF16, tag="ot", name="ot")
                if m % 2 == 0:
                    nc.vector.tensor_copy(ot[:], ps[:])
                else:
                    nc.scalar.activation(ot[:], ps[:], IDENT)
                deng = nc.sync if m % 2 == 0 else nc.scalar
                deng.dma_start(out_d[m * 128 : (m + 1) * 128, :], ot[:])
                if m + 4 < SQL // 128:
                    pss.append(emit_ident(m + 4))

    nc.compile()
    return nc


def _get_nc():
    if "nc" not in _CACHE:
        _CACHE["nc"] = _build_program()
    return _CACHE["nc"]


def make_in_maps(Q, K, V, Wq, Wk, Wv, Wout):
    import ml_dtypes

    BF = ml_dtypes.bfloat16

    def t(x):  # [r, c] fp32-ish -> bf16 [c, r]
        return np.asarray(x, dtype=np.float32).T.astype(BF)

    w4 = np.ascontiguousarray(
        np.concatenate(
            [
                t(np.asarray(w, dtype=np.float32).reshape(D, D))
                for w in (Wq, Wk, Wv, Wout)
            ],
            axis=1,
        )
    )
    Q = np.asarray(Q, dtype=np.float32)
    K = np.asarray(K, dtype=np.float32)
    V = np.asarray(V, dtype=np.float32)
    kv = [
        np.ascontiguousarray(np.concatenate([t(K[b]), t(V[b])], axis=1))
        for b in range(B)
    ]
    in_maps = []
    for c in range(N_CORES):
        b, qh = c // 2, c % 2
        in_maps.append(
            {
                "qt": np.ascontiguousarray(t(Q[b, qh * SQL : (qh + 1) * SQL, :])),
                "kv": kv[b],
                "w4": w4,
            }
        )
    return in_maps


def assemble_out(results):
    out = np.empty((B, S, D), dtype=np.float32)
    for c in range(N_CORES):
        b, qh = c // 2, c % 2
        out[b, qh * SQL : (qh + 1) * SQL, :] = np.asarray(
            results[c]["out"]
        ).astype(np.float32)
    return out


def kernel(Q, K, V, mask=None, Wq=None, Wk=None, Wv=None, Wout=None):
    # mask is a per-query additive constant before softmax -> softmax is
    # invariant to it; skipping it is numerically exact.
    from concourse.bass_utils import run_bass_kernel_spmd

    nc = _get_nc()
    in_maps = make_in_maps(Q, K, V, Wq, Wk, Wv, Wout)
    res = run_bass_kernel_spmd(nc, in_maps, core_ids=list(range(N_CORES)))
    return assemble_out(res.results)


if __name__ == "__main__":
    rng = np.random.default_rng(0)
    ins = {
        "Q": rng.standard_normal((B, S, D), dtype=np.float32),
        "K": rng.standard_normal((B, S, D), dtype=np.float32),
        "V": rng.standard_normal((B, S, D), dtype=np.float32),
        "mask": np.zeros((B, S), np.int32),
        "Wq": rng.standard_normal((H, DH, D), dtype=np.float32) / np.sqrt(D),
        "Wk": rng.standard_normal((H, DH, D), dtype=np.float32) / np.sqrt(D),
        "Wv": rng.standard_normal((H, DH, D), dtype=np.float32) / np.sqrt(D),
        "Wout": rng.standard_normal((D, D), dtype=np.float32) / np.sqrt(D),
    }
    out = kernel(**ins)
    print("out", out.shape, out.dtype, float(np.abs(out).max()))
